# revision 22
# baseline (speedup 1.0000x reference)
"""Trainium2 Bass kernel for per-pixel dot-product attention.

Reference op (per pixel, over C=80 channels split q/k/v = 8/64/8):
    qk[v] = sum_k q[k] * K[k, v] / sqrt(8)
    attn  = softmax(qk over v)
    out[v] = attn[v] * V[v]

Strategy: pure data-parallel over 8 NeuronCores — core i handles batch
i//2, H-rows half (i%2).  The op is memory-bound, so the kernel's whole
job is to stream HBM at line rate:

  * The host-side shard step converts the input to bf16 (the 2e-2
    correctness budget dwarfs bf16's ~0.4% quantization; measured
    rel-l2 ~5.6e-3) and packs it partition-major, chunk-major:
    x[p, (chunk, c, col)] so each chunk's load is one large DMA with
    C*ncol*2 (= tens of KB) contiguous bytes per partition.  That
    halves HBM traffic vs f32 (21 MB in + 2.1 MB out per core ≈ 65 us
    at the 358 GB/s per-core HBM limit) and makes every descriptor
    large.
  * On-device everything is elementwise on (128, ncol) pixel grids at
    bf16 (DVE 2x rate): products q[k]*K[k,v], an in-place pairwise
    add-tree over k, exp on ScalarE, softmax-denominator tree, and two
    output multiplies.  No PSUM / TensorE / GPSIMD.
  * Output is stored bf16 and upcast to f32 on the host.

Chunk columns are tapered (small first chunk = short DMA ramp-in,
small last chunk = short compute tail).
"""

import numpy as np
import ml_dtypes

NK = 8
NV = 8
C = NK + NK * NV + NV  # 80
B, H, W = 4, 512, 512
N_CORES = 8
ROWS = H // 2            # rows per core
PIX = ROWS * W           # pixels per core (131072)
COLS = PIX // 128        # columns per partition (1024)
_SCALE = 1.0 / float(np.sqrt(NK))
BF16 = ml_dtypes.bfloat16

CHUNK_COLS = [64, 128, 192, 256, 256, 128]


def _ensure_path():
    import sys
    p = "/opt/trn_rl_repo"
    if p not in sys.path:
        sys.path.insert(0, p)


def build_nc(chunk_cols=None, k_pieces=1, recip_on_act=True):
    """Per-core Bass program over a packed (128, C*COLS) bf16 shard.

    Chunk j occupies columns [C*off, C*(off+ncol)) of x, laid out
    (c, col) row-major per partition, so q/K/V loads are contiguous
    per-partition spans.  All DVE ops are bf16 so the 2x_1P perf mode
    engages; the reciprocal runs on ScalarE as exp(-ln s) (both live in
    the natural_log_exp_and_others table set — no table thrash).
    """
    _ensure_path()
    import concourse.tile as tile
    from concourse import bacc, mybir

    f32 = mybir.dt.float32
    bf = mybir.dt.bfloat16
    if chunk_cols is None:
        chunk_cols = CHUNK_COLS
    assert sum(chunk_cols) == COLS

    import bass_rust as _bass_rust
    from concourse.hw_specs import get_activation_tables

    class _Bacc(bacc.Bacc):
        """Resolve every activation to one table set: drop `exp` from the
        exp_and_others set so Exp and Ln both land in
        natural_log_exp_and_others — a single ACT_TABLE_LOAD instead of a
        ~2.7us thrash per chunk (set ids stay positional)."""

        def insert_act_table_loads(self):
            has_activation = any(
                isinstance(i, mybir.InstActivation)
                for b in self.main_func.blocks
                for i in b.instructions
            )
            if not has_activation:
                return
            exp_fn = mybir.ActivationFunctionType.Exp
            tables = []
            for name, funcs in get_activation_tables(self.m.arch).items():
                if name == "exp_and_others":
                    funcs = funcs - {exp_fn}
                tables.append((name, funcs))
            _bass_rust.insert_act_table_loads(self, tables)

    nc = _Bacc("TRN2", target_bir_lowering=False, debug=False)
    x = nc.dram_tensor("x", [128, C * COLS], bf, kind="ExternalInput")
    y = nc.dram_tensor("y", [128, NV * COLS], bf, kind="ExternalOutput")

    # deferred output DMAs: emit chunk j's store after chunk j+1's input
    # triggers so it never head-of-line-blocks loads on its ring
    pending_out = []

    def flush_out():
        for args in pending_out:
            nc.scalar.dma_start(**args)
        pending_out.clear()

    with tile.TileContext(nc) as tc:
        with (
            tc.tile_pool(name="inp", bufs=2) as in_pool,
            tc.tile_pool(name="work", bufs=2) as work_pool,
            tc.tile_pool(name="pipe", bufs=2) as pipe_pool,
        ):
            offs = []
            o = 0
            for ncol in chunk_cols:
                offs.append(o)
                o += ncol
            state = {}

            def emit_head(j):
                """DMA chunk j; DVE products + k-tree; ACT exp."""
                ncol, off = chunk_cols[j], offs[j]
                base = C * off
                q_t = in_pool.tile([128, NK * ncol], bf, name=f"q{j}", tag="q")
                k_t = in_pool.tile([128, NK * NV * ncol], bf, name=f"k{j}", tag="k")
                v_t = in_pool.tile([128, NV * ncol], bf, name=f"v{j}",
                                   tag="v", bufs=3)
                nc.sync.dma_start(out=q_t, in_=x[:, base:base + NK * ncol])
                kbase = base + NK * ncol
                # chunk 0 only: split K so the first products piece starts at
                # half-K; later chunks: one K DMA (measured fastest on Q1)
                pieces = k_pieces if j == 0 else 1
                sp = NK // pieces
                for h in range(pieces):
                    lo, hi = h * sp * NV * ncol, (h + 1) * sp * NV * ncol
                    nc.sync.dma_start(out=k_t[:, lo:hi],
                                      in_=x[:, kbase + lo:kbase + hi])
                nc.sync.dma_start(
                    out=v_t, in_=x[:, base + (NK + NK * NV) * ncol:base + C * ncol])
                # previous chunk's output, behind this chunk's input triggers
                flush_out()

                # prod[k,v] = q[k] * K[k,v]; broadcast q over v (stride-0
                # operand in in0 — a stride-0 in1 runs at half rate on DVE)
                prod = work_pool.tile([128, NK * NV * ncol], bf,
                                      name=f"prod{j}", tag="prod")
                p4 = prod.rearrange("p (k v x) -> p k v x", k=NK, v=NV)
                k4 = k_t.rearrange("p (k v x) -> p k v x", k=NK, v=NV)
                q_b = (q_t.rearrange("p (k x) -> p k x", k=NK)
                       .unsqueeze(2).broadcast_to((128, NK, NV, ncol)))
                for h in range(pieces):
                    kl, kh = h * sp, (h + 1) * sp
                    nc.vector.tensor_tensor(
                        p4[:, kl:kh], q_b[:, kl:kh], k4[:, kl:kh],
                        mybir.AluOpType.mult)
                # sum over k: in-place pairwise tree (strictly trailing
                # writes, single-engine serial on DVE)
                nc.vector.tensor_tensor(p4[:, 0:4], p4[:, 0:4], p4[:, 4:8],
                                        mybir.AluOpType.add)
                nc.vector.tensor_tensor(p4[:, 0:2], p4[:, 0:2], p4[:, 2:4],
                                        mybir.AluOpType.add)
                nc.vector.tensor_tensor(p4[:, 0:1], p4[:, 0:1], p4[:, 1:2],
                                        mybir.AluOpType.add)

                # e = exp(qk / sqrt(NK)) on ScalarE (runs while DVE moves on
                # to the next chunk's products — one-chunk software pipeline)
                e = pipe_pool.tile([128, NV * ncol], bf, name=f"e{j}", tag="e",
                                   bufs=3)
                nc.scalar.activation(e, prod[:, 0:NV * ncol],
                                     mybir.ActivationFunctionType.Exp,
                                     scale=_SCALE)
                state[j] = (e, v_t)

            def emit_tail_a(j):
                """Chunk j softmax, part 1 (needs e): denom tree, r on ACT,
                e*V multiply.  The r-dependent multiply is deferred to
                emit_tail_b (emitted after the NEXT chunk's products) so DVE
                never waits on the ACT ln/exp round trip."""
                ncol, off = chunk_cols[j], offs[j]
                e, v_t = state[j]
                # t-tree all in bf16 (sums of 8 positives — plenty of range)
                t1 = pipe_pool.tile([128, 4 * ncol], bf, name=f"t1{j}", tag="t1")
                with nc.allow_low_precision(reason="softmax denom in bf16"):
                    nc.vector.tensor_tensor(t1, e[:, 0:4 * ncol], e[:, 4 * ncol:],
                                            mybir.AluOpType.add)
                    nc.vector.tensor_tensor(t1[:, 0:2 * ncol], t1[:, 0:2 * ncol],
                                            t1[:, 2 * ncol:4 * ncol],
                                            mybir.AluOpType.add)
                    s = t1[:, 2 * ncol:3 * ncol]
                    nc.vector.tensor_tensor(s, t1[:, 0:ncol],
                                            t1[:, ncol:2 * ncol],
                                            mybir.AluOpType.add)
                r = pipe_pool.tile([128, ncol], bf, name=f"r{j}", tag="r")
                if recip_on_act:
                    # r = exp(-ln s) on ScalarE: off the DVE critical path,
                    # single act-table set (ln+exp coexist)
                    ls = pipe_pool.tile([128, ncol], f32, name=f"ls{j}", tag="ls")
                    nc.scalar.activation(ls, s, mybir.ActivationFunctionType.Ln)
                    nc.scalar.activation(r, ls, mybir.ActivationFunctionType.Exp,
                                         scale=-1.0)
                else:
                    rf = pipe_pool.tile([128, ncol], f32, name=f"rf{j}", tag="ls")
                    nc.vector.reciprocal(rf, s)
                    with nc.allow_low_precision(reason="r in bf16"):
                        nc.vector.tensor_copy(r, rf)

                # e *= V (does not depend on r; covers the ACT latency)
                e3 = e.rearrange("p (v x) -> p v x", v=NV)
                v3 = v_t.rearrange("p (v x) -> p v x", v=NV)
                nc.vector.tensor_tensor(e3, e3, v3, mybir.AluOpType.mult)
                state[j] = (e, r)

            def emit_tail_b(j):
                """Chunk j softmax, part 2: multiply by r, store (inline on
                the scalar ring — outputs never contend with sync-ring
                input loads)."""
                ncol, off = chunk_cols[j], offs[j]
                e, r = state.pop(j)
                e3 = e.rearrange("p (v x) -> p v x", v=NV)
                r_b = r.unsqueeze(1).broadcast_to((128, NV, ncol))
                nc.vector.tensor_tensor(e3, r_b, e3, mybir.AluOpType.mult)
                nc.scalar.dma_start(out=y[:, NV * off:NV * (off + ncol)], in_=e)

            # software pipeline, two-chunk skew on the r-multiply:
            #   head(0), head(1), tailA(0), head(2), tailB(0), tailA(1), ...
            # last two chunks fall back to adjacent tailA/tailB so their
            # output stores overlap the remaining compute
            n = len(chunk_cols)
            for j in range(n):
                emit_head(j)
                if j >= 2:
                    emit_tail_b(j - 2)
                if j >= 1 and j != n - 1:
                    emit_tail_a(j - 1)
            emit_tail_a(n - 2)
            emit_tail_b(n - 2)
            emit_tail_a(n - 1)
            emit_tail_b(n - 1)
            flush_out()
    nc.compile()
    return nc


_NC_CACHE = {}

BUILD_CFG = {
    "chunk_cols": CHUNK_COLS,
    "k_pieces": 2,
}


def _get_nc(**cfg):
    cfg = {**BUILD_CFG, **cfg}
    key = tuple(sorted(
        (k, tuple(v) if isinstance(v, list) else v) for k, v in cfg.items()
    ))
    if key not in _NC_CACHE:
        _NC_CACHE[key] = build_nc(**cfg)
    return _NC_CACHE[key]


def make_in_maps(inp, chunk_cols):
    """Shard + pack: core i gets batch i//2, H-half i%2, as a bf16
    (128, C*COLS) array laid out [p][chunk][c][col]."""
    in_maps = []
    for core in range(N_CORES):
        b, half = core // 2, core % 2
        shard = np.asarray(
            inp[b, :, half * ROWS:(half + 1) * ROWS, :], dtype=np.float32
        ).reshape(C, 128, COLS).astype(BF16)
        blocks = []
        off = 0
        for ncol in chunk_cols:
            blocks.append(
                np.ascontiguousarray(
                    shard[:, :, off:off + ncol].transpose(1, 0, 2)
                ).reshape(128, C * ncol))
            off += ncol
        in_maps.append({"x": np.concatenate(blocks, axis=1)})
    return in_maps


def assemble_out(results, chunk_cols):
    out = np.empty((B, NV, H, W), np.float32)
    for core in range(N_CORES):
        b, half = core // 2, core % 2
        yb = results[core]["y"]  # (128, NV*COLS) bf16, chunk-major
        blocks = []
        off = 0
        for ncol in chunk_cols:
            blocks.append(
                yb[:, NV * off:NV * (off + ncol)].reshape(128, NV, ncol))
            off += ncol
        full = np.concatenate(blocks, axis=2)            # (128, NV, COLS)
        out[b, :, half * ROWS:(half + 1) * ROWS, :] = (
            full.transpose(1, 0, 2).astype(np.float32).reshape(NV, ROWS, W))
    return out


def run_spmd(inp, trace=False, build_cfg=None, **kwargs):
    """Run the SPMD kernel on 8 cores; returns (full_output, BassKernelResults)."""
    _ensure_path()
    from concourse.bass_utils import run_bass_kernel_spmd

    inp = np.asarray(inp)
    assert inp.shape == (B, C, H, W), inp.shape
    cfg = {**BUILD_CFG, **(build_cfg or {})}
    nc = _get_nc(**cfg)
    res = run_bass_kernel_spmd(
        nc, make_in_maps(inp, cfg["chunk_cols"]), list(range(N_CORES)),
        trace=trace, **kwargs
    )
    return assemble_out(res.results, cfg["chunk_cols"]), res


def kernel(inp):
    out, _ = run_spmd(inp, trace=False)
    return out


# revision 23
# speedup vs baseline: 1.0120x; 1.0120x over previous
"""Trainium2 Bass kernel for per-pixel dot-product attention.

Reference op (per pixel, over C=80 channels split q/k/v = 8/64/8):
    qk[v] = sum_k q[k] * K[k, v] / sqrt(8)
    attn  = softmax(qk over v)
    out[v] = attn[v] * V[v]

Strategy: pure data-parallel over 8 NeuronCores — core i handles batch
i//2, H-rows half (i%2).  The op is memory-bound, so the kernel's whole
job is to stream HBM at line rate:

  * The host-side shard step converts the input to bf16 (the 2e-2
    correctness budget dwarfs bf16's ~0.4% quantization; measured
    rel-l2 ~5.6e-3) and packs it partition-major, chunk-major:
    x[p, (chunk, c, col)] so each chunk's load is one large DMA with
    C*ncol*2 (= tens of KB) contiguous bytes per partition.  That
    halves HBM traffic vs f32 (21 MB in + 2.1 MB out per core ≈ 65 us
    at the 358 GB/s per-core HBM limit) and makes every descriptor
    large.
  * On-device everything is elementwise on (128, ncol) pixel grids at
    bf16 (DVE 2x rate): products q[k]*K[k,v], an in-place pairwise
    add-tree over k, exp on ScalarE, softmax-denominator tree, and two
    output multiplies.  No PSUM / TensorE / GPSIMD.
  * Output is stored bf16 and upcast to f32 on the host.

Chunk columns are tapered (small first chunk = short DMA ramp-in,
small last chunk = short compute tail).
"""

import numpy as np
import ml_dtypes

NK = 8
NV = 8
C = NK + NK * NV + NV  # 80
B, H, W = 4, 512, 512
N_CORES = 8
ROWS = H // 2            # rows per core
PIX = ROWS * W           # pixels per core (131072)
COLS = PIX // 128        # columns per partition (1024)
_SCALE = 1.0 / float(np.sqrt(NK))
BF16 = ml_dtypes.bfloat16

CHUNK_COLS = [64, 128, 192, 256, 256, 128]


def _ensure_path():
    import sys
    p = "/opt/trn_rl_repo"
    if p not in sys.path:
        sys.path.insert(0, p)


def build_nc(chunk_cols=None, k_pieces=1, recip_on_act=True):
    """Per-core Bass program over a packed (128, C*COLS) bf16 shard.

    Chunk j occupies columns [C*off, C*(off+ncol)) of x, laid out
    (c, col) row-major per partition, so q/K/V loads are contiguous
    per-partition spans.  All DVE ops are bf16 so the 2x_1P perf mode
    engages; the reciprocal runs on ScalarE as exp(-ln s) (both live in
    the natural_log_exp_and_others table set — no table thrash).
    """
    _ensure_path()
    import concourse.tile as tile
    from concourse import bacc, mybir

    f32 = mybir.dt.float32
    bf = mybir.dt.bfloat16
    if chunk_cols is None:
        chunk_cols = CHUNK_COLS
    assert sum(chunk_cols) == COLS

    import bass_rust as _bass_rust
    from concourse.hw_specs import get_activation_tables

    class _Bacc(bacc.Bacc):
        """Resolve every activation to one table set: drop `exp` from the
        exp_and_others set so Exp and Ln both land in
        natural_log_exp_and_others — a single ACT_TABLE_LOAD instead of a
        ~2.7us thrash per chunk (set ids stay positional)."""

        def insert_act_table_loads(self):
            has_activation = any(
                isinstance(i, mybir.InstActivation)
                for b in self.main_func.blocks
                for i in b.instructions
            )
            if not has_activation:
                return
            exp_fn = mybir.ActivationFunctionType.Exp
            tables = []
            for name, funcs in get_activation_tables(self.m.arch).items():
                if name == "exp_and_others":
                    funcs = funcs - {exp_fn}
                tables.append((name, funcs))
            _bass_rust.insert_act_table_loads(self, tables)

    nc = _Bacc("TRN2", target_bir_lowering=False, debug=False)
    x = nc.dram_tensor("x", [128, C * COLS], bf, kind="ExternalInput")
    y = nc.dram_tensor("y", [128, NV * COLS], bf, kind="ExternalOutput")

    # deferred output DMAs: emit chunk j's store after chunk j+1's input
    # triggers so it never head-of-line-blocks loads on its ring
    pending_out = []

    def flush_out():
        for args in pending_out:
            nc.scalar.dma_start(**args)
        pending_out.clear()

    with tile.TileContext(nc) as tc:
        with (
            tc.tile_pool(name="inp", bufs=2) as in_pool,
            tc.tile_pool(name="work", bufs=2) as work_pool,
            tc.tile_pool(name="pipe", bufs=2) as pipe_pool,
        ):
            offs = []
            o = 0
            for ncol in chunk_cols:
                offs.append(o)
                o += ncol
            state = {}

            def emit_head(j):
                """DMA chunk j; DVE products + k-tree; ACT exp."""
                ncol, off = chunk_cols[j], offs[j]
                base = C * off
                q_t = in_pool.tile([128, NK * ncol], bf, name=f"q{j}", tag="q")
                k_t = in_pool.tile([128, NK * NV * ncol], bf, name=f"k{j}", tag="k",
                                   bufs=3)
                v_t = in_pool.tile([128, NV * ncol], bf, name=f"v{j}",
                                   tag="v", bufs=3)
                nc.sync.dma_start(out=q_t, in_=x[:, base:base + NK * ncol])
                kbase = base + NK * ncol
                # chunk 0 only: split K so the first products piece starts at
                # half-K; later chunks: one K DMA (measured fastest on Q1)
                pieces = k_pieces if j == 0 else 1
                sp = NK // pieces
                for h in range(pieces):
                    lo, hi = h * sp * NV * ncol, (h + 1) * sp * NV * ncol
                    nc.sync.dma_start(out=k_t[:, lo:hi],
                                      in_=x[:, kbase + lo:kbase + hi])
                nc.sync.dma_start(
                    out=v_t, in_=x[:, base + (NK + NK * NV) * ncol:base + C * ncol])
                # previous chunk's output, behind this chunk's input triggers
                flush_out()

                # prod[k,v] = q[k] * K[k,v]; broadcast q over v (stride-0
                # operand in in0 — a stride-0 in1 runs at half rate on DVE)
                prod = work_pool.tile([128, NK * NV * ncol], bf,
                                      name=f"prod{j}", tag="prod")
                p4 = prod.rearrange("p (k v x) -> p k v x", k=NK, v=NV)
                k4 = k_t.rearrange("p (k v x) -> p k v x", k=NK, v=NV)
                q_b = (q_t.rearrange("p (k x) -> p k x", k=NK)
                       .unsqueeze(2).broadcast_to((128, NK, NV, ncol)))
                for h in range(pieces):
                    kl, kh = h * sp, (h + 1) * sp
                    nc.vector.tensor_tensor(
                        p4[:, kl:kh], q_b[:, kl:kh], k4[:, kl:kh],
                        mybir.AluOpType.mult)
                # sum over k: in-place pairwise tree (strictly trailing
                # writes, single-engine serial on DVE)
                nc.vector.tensor_tensor(p4[:, 0:4], p4[:, 0:4], p4[:, 4:8],
                                        mybir.AluOpType.add)
                nc.vector.tensor_tensor(p4[:, 0:2], p4[:, 0:2], p4[:, 2:4],
                                        mybir.AluOpType.add)
                nc.vector.tensor_tensor(p4[:, 0:1], p4[:, 0:1], p4[:, 1:2],
                                        mybir.AluOpType.add)

                # e = exp(qk / sqrt(NK)) on ScalarE (runs while DVE moves on
                # to the next chunk's products — one-chunk software pipeline)
                e = pipe_pool.tile([128, NV * ncol], bf, name=f"e{j}", tag="e",
                                   bufs=3)
                nc.scalar.activation(e, prod[:, 0:NV * ncol],
                                     mybir.ActivationFunctionType.Exp,
                                     scale=_SCALE)
                state[j] = (e, v_t)

            def emit_tail_a(j):
                """Chunk j softmax, part 1 (needs e): denom tree, r on ACT,
                e*V multiply.  The r-dependent multiply is deferred to
                emit_tail_b (emitted after the NEXT chunk's products) so DVE
                never waits on the ACT ln/exp round trip."""
                ncol, off = chunk_cols[j], offs[j]
                e, v_t = state[j]
                # t-tree all in bf16 (sums of 8 positives — plenty of range)
                t1 = pipe_pool.tile([128, 4 * ncol], bf, name=f"t1{j}", tag="t1")
                with nc.allow_low_precision(reason="softmax denom in bf16"):
                    nc.vector.tensor_tensor(t1, e[:, 0:4 * ncol], e[:, 4 * ncol:],
                                            mybir.AluOpType.add)
                    nc.vector.tensor_tensor(t1[:, 0:2 * ncol], t1[:, 0:2 * ncol],
                                            t1[:, 2 * ncol:4 * ncol],
                                            mybir.AluOpType.add)
                    s = t1[:, 2 * ncol:3 * ncol]
                    nc.vector.tensor_tensor(s, t1[:, 0:ncol],
                                            t1[:, ncol:2 * ncol],
                                            mybir.AluOpType.add)
                r = pipe_pool.tile([128, ncol], bf, name=f"r{j}", tag="r")
                if recip_on_act:
                    # r = exp(-ln s) on ScalarE: off the DVE critical path,
                    # single act-table set (ln+exp coexist)
                    ls = pipe_pool.tile([128, ncol], f32, name=f"ls{j}", tag="ls")
                    nc.scalar.activation(ls, s, mybir.ActivationFunctionType.Ln)
                    nc.scalar.activation(r, ls, mybir.ActivationFunctionType.Exp,
                                         scale=-1.0)
                else:
                    rf = pipe_pool.tile([128, ncol], f32, name=f"rf{j}", tag="ls")
                    nc.vector.reciprocal(rf, s)
                    with nc.allow_low_precision(reason="r in bf16"):
                        nc.vector.tensor_copy(r, rf)

                # e *= V (does not depend on r; covers the ACT latency)
                e3 = e.rearrange("p (v x) -> p v x", v=NV)
                v3 = v_t.rearrange("p (v x) -> p v x", v=NV)
                nc.vector.tensor_tensor(e3, e3, v3, mybir.AluOpType.mult)
                state[j] = (e, r)

            def emit_tail_b(j):
                """Chunk j softmax, part 2: multiply by r, store (inline on
                the scalar ring — outputs never contend with sync-ring
                input loads)."""
                ncol, off = chunk_cols[j], offs[j]
                e, r = state.pop(j)
                e3 = e.rearrange("p (v x) -> p v x", v=NV)
                r_b = r.unsqueeze(1).broadcast_to((128, NV, ncol))
                nc.vector.tensor_tensor(e3, r_b, e3, mybir.AluOpType.mult)
                nc.scalar.dma_start(out=y[:, NV * off:NV * (off + ncol)], in_=e)

            # software pipeline, two-chunk skew on the r-multiply:
            #   head(0), head(1), tailA(0), head(2), tailB(0), tailA(1), ...
            # last two chunks fall back to adjacent tailA/tailB so their
            # output stores overlap the remaining compute
            n = len(chunk_cols)
            for j in range(n):
                emit_head(j)
                if j >= 2:
                    emit_tail_b(j - 2)
                if j >= 1 and j != n - 1:
                    emit_tail_a(j - 1)
            emit_tail_a(n - 2)
            emit_tail_b(n - 2)
            emit_tail_a(n - 1)
            emit_tail_b(n - 1)
            flush_out()
    nc.compile()
    return nc


_NC_CACHE = {}

BUILD_CFG = {
    "chunk_cols": CHUNK_COLS,
    "k_pieces": 2,
}


def _get_nc(**cfg):
    cfg = {**BUILD_CFG, **cfg}
    key = tuple(sorted(
        (k, tuple(v) if isinstance(v, list) else v) for k, v in cfg.items()
    ))
    if key not in _NC_CACHE:
        _NC_CACHE[key] = build_nc(**cfg)
    return _NC_CACHE[key]


def make_in_maps(inp, chunk_cols):
    """Shard + pack: core i gets batch i//2, H-half i%2, as a bf16
    (128, C*COLS) array laid out [p][chunk][c][col]."""
    in_maps = []
    for core in range(N_CORES):
        b, half = core // 2, core % 2
        shard = np.asarray(
            inp[b, :, half * ROWS:(half + 1) * ROWS, :], dtype=np.float32
        ).reshape(C, 128, COLS).astype(BF16)
        blocks = []
        off = 0
        for ncol in chunk_cols:
            blocks.append(
                np.ascontiguousarray(
                    shard[:, :, off:off + ncol].transpose(1, 0, 2)
                ).reshape(128, C * ncol))
            off += ncol
        in_maps.append({"x": np.concatenate(blocks, axis=1)})
    return in_maps


def assemble_out(results, chunk_cols):
    out = np.empty((B, NV, H, W), np.float32)
    for core in range(N_CORES):
        b, half = core // 2, core % 2
        yb = results[core]["y"]  # (128, NV*COLS) bf16, chunk-major
        blocks = []
        off = 0
        for ncol in chunk_cols:
            blocks.append(
                yb[:, NV * off:NV * (off + ncol)].reshape(128, NV, ncol))
            off += ncol
        full = np.concatenate(blocks, axis=2)            # (128, NV, COLS)
        out[b, :, half * ROWS:(half + 1) * ROWS, :] = (
            full.transpose(1, 0, 2).astype(np.float32).reshape(NV, ROWS, W))
    return out


def run_spmd(inp, trace=False, build_cfg=None, **kwargs):
    """Run the SPMD kernel on 8 cores; returns (full_output, BassKernelResults)."""
    _ensure_path()
    from concourse.bass_utils import run_bass_kernel_spmd

    inp = np.asarray(inp)
    assert inp.shape == (B, C, H, W), inp.shape
    cfg = {**BUILD_CFG, **(build_cfg or {})}
    nc = _get_nc(**cfg)
    res = run_bass_kernel_spmd(
        nc, make_in_maps(inp, cfg["chunk_cols"]), list(range(N_CORES)),
        trace=trace, **kwargs
    )
    return assemble_out(res.results, cfg["chunk_cols"]), res


def kernel(inp):
    out, _ = run_spmd(inp, trace=False)
    return out


# revision 25
# speedup vs baseline: 1.0198x; 1.0077x over previous
"""Trainium2 Bass kernel for per-pixel dot-product attention.

Reference op (per pixel, over C=80 channels split q/k/v = 8/64/8):
    qk[v] = sum_k q[k] * K[k, v] / sqrt(8)
    attn  = softmax(qk over v)
    out[v] = attn[v] * V[v]

Strategy: pure data-parallel over 8 NeuronCores — core i handles batch
i//2, H-rows half (i%2).  The op is memory-bound, so the kernel's whole
job is to stream HBM at line rate:

  * The host-side shard step converts the input to bf16 (the 2e-2
    correctness budget dwarfs bf16's ~0.4% quantization; measured
    rel-l2 ~5.6e-3) and packs it partition-major, chunk-major:
    x[p, (chunk, c, col)] so each chunk's load is one large DMA with
    C*ncol*2 (= tens of KB) contiguous bytes per partition.  That
    halves HBM traffic vs f32 (21 MB in + 2.1 MB out per core ≈ 65 us
    at the 358 GB/s per-core HBM limit) and makes every descriptor
    large.
  * On-device everything is elementwise on (128, ncol) pixel grids at
    bf16 (DVE 2x_1P rate, all ops verified at 2x in the trace):
    products q[k]*K[k,v], an in-place pairwise add-tree over k, exp on
    ScalarE, softmax-denominator tree, and two output multiplies.  No
    PSUM / TensorE (per-pixel weights can't matmul) / GPSIMD (every
    DVE tensor_tensor holds the shared SBUF port pair, fully blocking
    Pool).
  * Output is stored bf16 and upcast to f32 on the host.

After the bf16 switch the kernel is DVE-bound, not DMA-bound: the
softmax pipeline is 143 bf16 elements/pixel = ~85 us of DVE busy at
the 245 G elem/s 2x rate, vs ~57 us of input DMA.  The remaining
structure is scheduling: a one-chunk software-pipeline skew (chunk
j+1's products run between chunk j's exp and its softmax tail) plus a
two-chunk skew on the r-multiply so DVE never waits on the ACT ln/exp
round trip (r = exp(-ln s) on ScalarE; ln+exp share one act-table set
— the Bacc subclass below pins Exp to it to avoid a per-chunk ~2.7us
table thrash).  Chunk columns are tapered: small first chunk = short
DMA ramp-in, small last chunk = short compute tail.

Measured on trn2 (8 cores): 105.5 us/NEFF, rel-l2 5.6e-3 vs the f32
reference (baseline f32 kernel: 161 us; DVE-busy floor ~85 us + 7 us
fixed NEFF preamble + ramp/tail).
"""

import numpy as np
import ml_dtypes

NK = 8
NV = 8
C = NK + NK * NV + NV  # 80
B, H, W = 4, 512, 512
N_CORES = 8
ROWS = H // 2            # rows per core
PIX = ROWS * W           # pixels per core (131072)
COLS = PIX // 128        # columns per partition (1024)
_SCALE = 1.0 / float(np.sqrt(NK))
BF16 = ml_dtypes.bfloat16

CHUNK_COLS = [64, 128, 192, 256, 256, 128]


def _ensure_path():
    import sys
    p = "/opt/trn_rl_repo"
    if p not in sys.path:
        sys.path.insert(0, p)


def build_nc(chunk_cols=None, k_pieces=1, recip_on_act=True):
    """Per-core Bass program over a packed (128, C*COLS) bf16 shard.

    Chunk j occupies columns [C*off, C*(off+ncol)) of x, laid out
    (c, col) row-major per partition, so q/K/V loads are contiguous
    per-partition spans.  All DVE ops are bf16 so the 2x_1P perf mode
    engages; the reciprocal runs on ScalarE as exp(-ln s) (both live in
    the natural_log_exp_and_others table set — no table thrash).
    """
    _ensure_path()
    import concourse.tile as tile
    from concourse import bacc, mybir

    f32 = mybir.dt.float32
    bf = mybir.dt.bfloat16
    if chunk_cols is None:
        chunk_cols = CHUNK_COLS
    assert sum(chunk_cols) == COLS

    import bass_rust as _bass_rust
    from concourse.hw_specs import get_activation_tables

    class _Bacc(bacc.Bacc):
        """Resolve every activation to one table set: drop `exp` from the
        exp_and_others set so Exp and Ln both land in
        natural_log_exp_and_others — a single ACT_TABLE_LOAD instead of a
        ~2.7us thrash per chunk (set ids stay positional)."""

        def insert_act_table_loads(self):
            has_activation = any(
                isinstance(i, mybir.InstActivation)
                for b in self.main_func.blocks
                for i in b.instructions
            )
            if not has_activation:
                return
            exp_fn = mybir.ActivationFunctionType.Exp
            tables = []
            for name, funcs in get_activation_tables(self.m.arch).items():
                if name == "exp_and_others":
                    funcs = funcs - {exp_fn}
                tables.append((name, funcs))
            _bass_rust.insert_act_table_loads(self, tables)

    nc = _Bacc("TRN2", target_bir_lowering=False, debug=False)
    x = nc.dram_tensor("x", [128, C * COLS], bf, kind="ExternalInput")
    y = nc.dram_tensor("y", [128, NV * COLS], bf, kind="ExternalOutput")

    with tile.TileContext(nc) as tc:
        with (
            tc.tile_pool(name="inp", bufs=2) as in_pool,
            tc.tile_pool(name="work", bufs=2) as work_pool,
            tc.tile_pool(name="pipe", bufs=2) as pipe_pool,
        ):
            offs = []
            o = 0
            for ncol in chunk_cols:
                offs.append(o)
                o += ncol
            state = {}

            def emit_head(j):
                """DMA chunk j; DVE products + k-tree; ACT exp."""
                ncol, off = chunk_cols[j], offs[j]
                base = C * off
                q_t = in_pool.tile([128, NK * ncol], bf, name=f"q{j}", tag="q")
                k_t = in_pool.tile([128, NK * NV * ncol], bf, name=f"k{j}", tag="k")
                v_t = in_pool.tile([128, NV * ncol], bf, name=f"v{j}",
                                   tag="v", bufs=3)
                nc.sync.dma_start(out=q_t, in_=x[:, base:base + NK * ncol])
                kbase = base + NK * ncol
                # chunk 0 only: split K so the first products piece starts at
                # half-K; later chunks: one K DMA (measured fastest on Q1)
                pieces = k_pieces if j == 0 else 1
                sp = NK // pieces
                for h in range(pieces):
                    lo, hi = h * sp * NV * ncol, (h + 1) * sp * NV * ncol
                    nc.sync.dma_start(out=k_t[:, lo:hi],
                                      in_=x[:, kbase + lo:kbase + hi])
                nc.sync.dma_start(
                    out=v_t, in_=x[:, base + (NK + NK * NV) * ncol:base + C * ncol])

                # prod[k,v] = q[k] * K[k,v]; broadcast q over v (stride-0
                # operand in in0 — a stride-0 in1 runs at half rate on DVE)
                prod = work_pool.tile([128, NK * NV * ncol], bf,
                                      name=f"prod{j}", tag="prod")
                p4 = prod.rearrange("p (k v x) -> p k v x", k=NK, v=NV)
                k4 = k_t.rearrange("p (k v x) -> p k v x", k=NK, v=NV)
                q_b = (q_t.rearrange("p (k x) -> p k x", k=NK)
                       .unsqueeze(2).broadcast_to((128, NK, NV, ncol)))
                for h in range(pieces):
                    kl, kh = h * sp, (h + 1) * sp
                    nc.vector.tensor_tensor(
                        p4[:, kl:kh], q_b[:, kl:kh], k4[:, kl:kh],
                        mybir.AluOpType.mult)
                # sum over k: in-place pairwise tree (strictly trailing
                # writes, single-engine serial on DVE)
                nc.vector.tensor_tensor(p4[:, 0:4], p4[:, 0:4], p4[:, 4:8],
                                        mybir.AluOpType.add)
                nc.vector.tensor_tensor(p4[:, 0:2], p4[:, 0:2], p4[:, 2:4],
                                        mybir.AluOpType.add)
                nc.vector.tensor_tensor(p4[:, 0:1], p4[:, 0:1], p4[:, 1:2],
                                        mybir.AluOpType.add)

                # e = exp(qk / sqrt(NK)) on ScalarE (runs while DVE moves on
                # to the next chunk's products — one-chunk software pipeline)
                e = pipe_pool.tile([128, NV * ncol], bf, name=f"e{j}", tag="e",
                                   bufs=3)
                nc.scalar.activation(e, prod[:, 0:NV * ncol],
                                     mybir.ActivationFunctionType.Exp,
                                     scale=_SCALE)
                state[j] = (e, v_t)

            def emit_tail_a(j):
                """Chunk j softmax, part 1 (needs e): denom tree, r on ACT,
                e*V multiply.  The r-dependent multiply is deferred to
                emit_tail_b (emitted after the NEXT chunk's products) so DVE
                never waits on the ACT ln/exp round trip."""
                ncol, off = chunk_cols[j], offs[j]
                e, v_t = state[j]
                # t-tree all in bf16 (sums of 8 positives — plenty of range)
                t1 = pipe_pool.tile([128, 4 * ncol], bf, name=f"t1{j}", tag="t1")
                with nc.allow_low_precision(reason="softmax denom in bf16"):
                    nc.vector.tensor_tensor(t1, e[:, 0:4 * ncol], e[:, 4 * ncol:],
                                            mybir.AluOpType.add)
                    nc.vector.tensor_tensor(t1[:, 0:2 * ncol], t1[:, 0:2 * ncol],
                                            t1[:, 2 * ncol:4 * ncol],
                                            mybir.AluOpType.add)
                    s = t1[:, 2 * ncol:3 * ncol]
                    nc.vector.tensor_tensor(s, t1[:, 0:ncol],
                                            t1[:, ncol:2 * ncol],
                                            mybir.AluOpType.add)
                r = pipe_pool.tile([128, ncol], bf, name=f"r{j}", tag="r")
                if recip_on_act:
                    # r = exp(-ln s) on ScalarE: off the DVE critical path,
                    # single act-table set (ln+exp coexist)
                    ls = pipe_pool.tile([128, ncol], f32, name=f"ls{j}", tag="ls")
                    nc.scalar.activation(ls, s, mybir.ActivationFunctionType.Ln)
                    nc.scalar.activation(r, ls, mybir.ActivationFunctionType.Exp,
                                         scale=-1.0)
                else:
                    rf = pipe_pool.tile([128, ncol], f32, name=f"rf{j}", tag="ls")
                    nc.vector.reciprocal(rf, s)
                    with nc.allow_low_precision(reason="r in bf16"):
                        nc.vector.tensor_copy(r, rf)

                # e *= V (does not depend on r; covers the ACT latency)
                e3 = e.rearrange("p (v x) -> p v x", v=NV)
                v3 = v_t.rearrange("p (v x) -> p v x", v=NV)
                nc.vector.tensor_tensor(e3, e3, v3, mybir.AluOpType.mult)
                state[j] = (e, r)

            def emit_tail_b(j):
                """Chunk j softmax, part 2: multiply by r, store (inline on
                the scalar ring — outputs never contend with sync-ring
                input loads)."""
                ncol, off = chunk_cols[j], offs[j]
                e, r = state.pop(j)
                e3 = e.rearrange("p (v x) -> p v x", v=NV)
                r_b = r.unsqueeze(1).broadcast_to((128, NV, ncol))
                nc.vector.tensor_tensor(e3, r_b, e3, mybir.AluOpType.mult)
                nc.scalar.dma_start(out=y[:, NV * off:NV * (off + ncol)], in_=e)

            # software pipeline, two-chunk skew on the r-multiply:
            #   head(0), head(1), tailA(0), head(2), tailB(0), tailA(1), ...
            # last two chunks fall back to adjacent tailA/tailB so their
            # output stores overlap the remaining compute
            n = len(chunk_cols)
            for j in range(n):
                emit_head(j)
                if j >= 2:
                    emit_tail_b(j - 2)
                if j >= 1 and j != n - 1:
                    emit_tail_a(j - 1)
            emit_tail_a(n - 2)
            emit_tail_b(n - 2)
            emit_tail_a(n - 1)
            emit_tail_b(n - 1)
    nc.compile()
    return nc


_NC_CACHE = {}

BUILD_CFG = {
    "chunk_cols": CHUNK_COLS,
    "k_pieces": 2,
}


def _get_nc(**cfg):
    cfg = {**BUILD_CFG, **cfg}
    key = tuple(sorted(
        (k, tuple(v) if isinstance(v, list) else v) for k, v in cfg.items()
    ))
    if key not in _NC_CACHE:
        _NC_CACHE[key] = build_nc(**cfg)
    return _NC_CACHE[key]


def make_in_maps(inp, chunk_cols):
    """Shard + pack: core i gets batch i//2, H-half i%2, as a bf16
    (128, C*COLS) array laid out [p][chunk][c][col]."""
    in_maps = []
    for core in range(N_CORES):
        b, half = core // 2, core % 2
        shard = np.asarray(
            inp[b, :, half * ROWS:(half + 1) * ROWS, :], dtype=np.float32
        ).reshape(C, 128, COLS).astype(BF16)
        blocks = []
        off = 0
        for ncol in chunk_cols:
            blocks.append(
                np.ascontiguousarray(
                    shard[:, :, off:off + ncol].transpose(1, 0, 2)
                ).reshape(128, C * ncol))
            off += ncol
        in_maps.append({"x": np.concatenate(blocks, axis=1)})
    return in_maps


def assemble_out(results, chunk_cols):
    out = np.empty((B, NV, H, W), np.float32)
    for core in range(N_CORES):
        b, half = core // 2, core % 2
        yb = results[core]["y"]  # (128, NV*COLS) bf16, chunk-major
        blocks = []
        off = 0
        for ncol in chunk_cols:
            blocks.append(
                yb[:, NV * off:NV * (off + ncol)].reshape(128, NV, ncol))
            off += ncol
        full = np.concatenate(blocks, axis=2)            # (128, NV, COLS)
        out[b, :, half * ROWS:(half + 1) * ROWS, :] = (
            full.transpose(1, 0, 2).astype(np.float32).reshape(NV, ROWS, W))
    return out


def run_spmd(inp, trace=False, build_cfg=None, **kwargs):
    """Run the SPMD kernel on 8 cores; returns (full_output, BassKernelResults)."""
    _ensure_path()
    from concourse.bass_utils import run_bass_kernel_spmd

    inp = np.asarray(inp)
    assert inp.shape == (B, C, H, W), inp.shape
    cfg = {**BUILD_CFG, **(build_cfg or {})}
    nc = _get_nc(**cfg)
    res = run_bass_kernel_spmd(
        nc, make_in_maps(inp, cfg["chunk_cols"]), list(range(N_CORES)),
        trace=trace, **kwargs
    )
    return assemble_out(res.results, cfg["chunk_cols"]), res


def kernel(inp):
    out, _ = run_spmd(inp, trace=False)
    return out
